# revision 63
# baseline (speedup 1.0000x reference)
"""Bass/Tile kernel for nn_CustomCrossAttnProcessor (8-core data-parallel).

Each NeuronCore processes one batch element (B=8 == n_cores).

v2 redesign (vs baseline):
  - All transposes moved off the PE: hs -> hsT and hs_sum -> hsT2 go through
    DMA xbar transpose (bf16, 16x128 tiles); Pool converts hs fp32->bf16.
  - Scores paired: one [CAT,512] psum tile holds two heads (two single-shot
    matmuls), one exp per pair (halves ACT op count on the exp path).
  - Out-projection: bias matmul removed (DVE adds a pre-broadcast bias tile
    during the psum->sbuf drain); 12 half-chunks interleave into the score
    and PV phases of the *second-next* block (software pipeline deepened so
    the PE never waits on the DVE stats chain).
  - PV tiles and q-projection accumulation groups interleave so DVE
    normalize latency hides under PE work.
"""
import sys

for _p in ("/opt/trn_rl_repo",):
    if _p not in sys.path:
        sys.path.append(_p)

from contextlib import ExitStack

import numpy as np

import concourse.bass as bass  # noqa: F401
import concourse.tile as tile
import concourse.mybir as mybir
from concourse import bass_utils, bacc
from concourse.bass import ts, ds
from concourse.masks import make_identity

B, S, D = 8, 4096, 1280
T, P_IP, C = 77, 16, 2048
H, HD = 20, 64
SB = 256            # tokens per s-block
NBLK = S // SB      # 16
SCALE = HD ** -0.5  # 0.125
EPS = 1e-7
KD = D // 128       # 10
KC = C // 128       # 16
CAT = 112           # rows: txt [0:77], gap [77:96], ip [96:112]
IPOFF = 96
VW = HD + 2         # 66: v cols + ones col (softmax sum) + v-rowsum col
FILLD = 3           # q-proj pipeline depth
ALU = mybir.AluOpType
FT = mybir.ActivationFunctionType

f32 = mybir.dt.float32
f32r = mybir.dt.float32r
bf16 = mybir.dt.bfloat16

_CACHE = {}


def _build():
    nc = bacc.Bacc(
        "TRN2", target_bir_lowering=False, debug=False, enable_asserts=False,
        num_devices=8,
    )
    hs_d = nc.dram_tensor("hidden_states", [S, D], f32, kind="ExternalInput").ap()
    enc_d = nc.dram_tensor("encoder_hidden_states", [T, C], f32,
                           kind="ExternalInput").ap()
    ip_d = nc.dram_tensor("ip_hidden_states", [P_IP, C], f32,
                          kind="ExternalInput").ap()
    wq_d = nc.dram_tensor("w_q", [D, D], f32r, kind="ExternalInput").ap()
    wk_d = nc.dram_tensor("w_k", [C, D], f32r, kind="ExternalInput").ap()
    wv_d = nc.dram_tensor("w_v", [C, D], f32r, kind="ExternalInput").ap()
    wkip_d = nc.dram_tensor("w_k_ip", [C, D], f32r, kind="ExternalInput").ap()
    wvip_d = nc.dram_tensor("w_v_ip", [C, D], f32r, kind="ExternalInput").ap()
    wout_d = nc.dram_tensor("w_out", [D, D], f32, kind="ExternalInput").ap()
    bout_d = nc.dram_tensor("b_out", [D], f32, kind="ExternalInput").ap()
    out_d = nc.dram_tensor("out", [S, D], f32, kind="ExternalOutput").ap()

    with tile.TileContext(nc) as tc, ExitStack() as ctx:
        n = tc.nc
        const = ctx.enter_context(tc.tile_pool(name="const", bufs=1))
        wq_sb = const.tile([128, KD, D], bf16)
        wout_bf = const.tile([128, KD, D], bf16)
        ktc_sb = const.tile([128, KD, CAT], bf16)
        vcat = const.tile([CAT, H, VW], bf16)
        bias_full = const.tile([128, D], f32)

        n.gpsimd.memset(ktc_sb[:, :, T:IPOFF], 0.0)
        n.vector.memset(vcat[0:T, :, HD:HD + 1], 1.0)
        n.vector.memset(vcat[IPOFF:CAT, :, HD:HD + 1], 1.0)

        # ---------------- loop pools -------------------------------------
        lp = ctx.enter_context(tc.tile_pool(name="lp", bufs=3))      # hs f32
        lpb = ctx.enter_context(tc.tile_pool(name="lpb", bufs=2))    # hs bf16
        lph = ctx.enter_context(tc.tile_pool(name="lph", bufs=2))    # hsT
        lpq = ctx.enter_context(tc.tile_pool(name="lpq", bufs=FILLD))  # qT
        lscr = ctx.enter_context(tc.tile_pool(name="lscr", bufs=1))  # sq scratch
        lp1 = ctx.enter_context(tc.tile_pool(name="lp1", bufs=2))    # lat/ipo
        lph2 = ctx.enter_context(tc.tile_pool(name="lph2", bufs=2))  # hsT2
        lps = ctx.enter_context(tc.tile_pool(name="lps", bufs=2))    # stats
        lpo = ctx.enter_context(tc.tile_pool(name="lpo", bufs=2))    # ost
        lpp = ctx.enter_context(tc.tile_pool(name="lpp", bufs=20))   # pT
        # psum: acc ring ([128,512]: q-proj groups, out-proj chunks, setup
        # transposes). scores + pv rings alloc'd after setup (bank budget:
        # setup kvp needs 5 banks alongside acc's 3).
        ps_acc = ctx.enter_context(tc.tile_pool(name="ps_acc", bufs=3,
                                                space="PSUM"))

        hs_tiles = {}

        def emit_load(b):
            if b >= NBLK:
                return
            for si in range(2):
                t_ = lp.tile([128, D], f32, tag="hs", name=f"hs{b}_{si}")
                n.sync.dma_start(t_[:], hs_d[ds(b * SB + si * 128, 128), :])
                hs_tiles[(b, si)] = t_

        qT_tiles = {}
        hsT_tiles = {}

        def emit_fill_start(b):
            """hs fp32 -> bf16 (Pool), then DMA xbar transpose -> hsT."""
            if b >= NBLK:
                return
            hsT = lph.tile([128, KD, SB], bf16, tag="hsT", name=f"hsT{b}")
            for si in range(2):
                hsb = lpb.tile([128, D], bf16, tag="hsb")
                n.gpsimd.tensor_copy(hsb[:], hs_tiles.pop((b, si))[:])
                n.sync.dma_start(hsT[:, :, ds(si * 128, 128)], hsb[:],
                                 transpose=True)
            hsT_tiles[b] = hsT

        def make_qp_closures(b):
            """10 closures: q-projection for block b in (dp, dd) groups of
            10 accumulating matmuls each; qT drain (ACT) after each dd pair."""
            if b >= NBLK:
                return []
            hsT = hsT_tiles.pop(b)
            qT = lpq.tile([128, KD, SB], bf16, tag="qT", name=f"qT{b}")
            qT_tiles[b] = qT
            out = []
            state = {}

            def mk(dp, dd):
                def go():
                    if dd == 0:
                        state["qp"] = ps_acc.tile([128, 512], f32, tag="acc",
                                                  name=f"qp{b}_{dp}")
                    qp = state["qp"]
                    for k in range(KD):
                        n.tensor.matmul(qp[:, ds(dd * SB, SB)],
                                        wq_sb[:, k, ts(dp + dd, 128)],
                                        hsT[:, k, :], start=(k == 0),
                                        stop=(k == KD - 1))
                    if dd == 1:
                        n.scalar.activation(
                            qT[:, dp:dp + 2, :].rearrange("p a b -> p (a b)"),
                            qp[:], FT.Copy)
                return go

            for dp in range(0, KD, 2):
                for dd in range(2):
                    out.append(mk(dp, dd))
            return out

        def emit_fill_start_pe(b, ident):
            """setup-only fill start: PE transposes of fp32 hs (PE is idle
            during the DMA-bound setup; keeps the fill off the serial DMA
            queue, where a not-yet-ready xbar transpose head-of-line blocks
            the weight stream)."""
            hsT = lph.tile([128, KD, SB], bf16, tag="hsT", name=f"hsT{b}")
            for si in range(2):
                hst = hs_tiles.pop((b, si))
                for g0 in range(0, KD, 4):
                    kk = min(4, KD - g0)
                    tp = ps_acc.tile([128, 512], f32, tag="acc",
                                     name=f"tp{b}_{si}_{g0}")
                    for j in range(kk):
                        n.tensor.transpose(tp[:, ds(j * 128, 128)],
                                           hst[:, ts(g0 + j, 128)], ident[:])
                    n.scalar.activation(
                        hsT[:, g0:g0 + kk, ds(si * 128, 128)],
                        tp[:, 0:kk * 128].rearrange("p (a b) -> p a b", b=128),
                        FT.Copy)
            hsT_tiles[b] = hsT

        def emit_fill(b, ident):
            """setup-time fill: PE-transpose start + all qp groups inline."""
            if b >= NBLK:
                return
            emit_fill_start_pe(b, ident)
            for c in make_qp_closures(b):
                c()

        fills = list(range(FILLD))

        def maybe_fill(ident):
            if fills:
                b = fills.pop(0)
                emit_fill(b, ident)
                if b + 2 < FILLD:
                    emit_load(b + 2)  # later loads defer past the weights

        # ================= setup ===========================================
        with tc.tile_pool(name="setup", bufs=1) as setup, \
             tc.tile_pool(name="setup_w", bufs=2) as setup_w:
            ident = setup.tile([128, 128], f32, tag="ident")
            make_identity(n, ident[:])
            ones_bf = setup.tile([1, 128], bf16, tag="ones")
            bias_bf = setup.tile([1, D], bf16, tag="biasbf")
            n.gpsimd.memset(ones_bf[:], 1.0)
            encT = setup.tile([128, KC, T], f32r, tag="encT")
            ipT = setup.tile([128, KC, P_IP], f32r, tag="ipT")
            with tc.tile_pool(name="setup_e", bufs=1) as setup_e:
                enc_sb = setup_e.tile([CAT, C], f32, tag="enc")
                n.sync.dma_start(enc_sb[0:T, :], enc_d)
                n.sync.dma_start(enc_sb[IPOFF:CAT, :], ip_d)
                n.sync.dma_start(bias_full[0:1, :], bout_d[None, :])
                # stage + convert w_q chunks to bf16 (Pool)
                wqf = wq_d.bitcast(f32).rearrange("(ko ki) m -> ki ko m",
                                                  ki=128)
                for k in range(KD):
                    wq_a = setup_w.tile([128, 1024], f32, tag="wk1024")
                    n.sync.dma_start(wq_a[:], wqf[:, k, 0:1024])
                    wq_b = setup_w.tile([128, 256], f32, tag="w256")
                    n.sync.dma_start(wq_b[:], wqf[:, k, 1024:1280])
                    n.gpsimd.tensor_copy(wq_sb[:, k, 0:1024], wq_a[:])
                    n.gpsimd.tensor_copy(wq_sb[:, k, 1024:1280], wq_b[:])
                emit_load(0)
                emit_load(1)
                n.gpsimd.tensor_copy(bias_bf[:], bias_full[0:1, :])
                # bias broadcast to all 128 partitions via PE outer product
                for j in range(3):
                    w = min(512, D - j * 512)
                    pb = ps_acc.tile([128, 512], f32, tag="acc")
                    n.tensor.matmul(pb[:, :w], ones_bf[:],
                                    bias_bf[:, ds(j * 512, w)],
                                    start=True, stop=True)
                    n.vector.tensor_copy(bias_full[:, ds(j * 512, w)],
                                         pb[:, :w])
                for c in range(KC):
                    tpe = ps_acc.tile([128, 512], f32, tag="acc")
                    n.tensor.transpose(tpe[:, 0:T], enc_sb[0:T, ts(c, 128)],
                                       ident[:T, :T])
                    n.tensor.transpose(tpe[:, 128:128 + P_IP],
                                       enc_sb[IPOFF:CAT, ts(c, 128)],
                                       ident[IPOFF:CAT, IPOFF:IPOFF + P_IP],
                                       tile_position=(IPOFF, 0))
                    n.vector.tensor_copy(encT[:, c, :], tpe[:, 0:T])
                    n.vector.tensor_copy(ipT[:, c, :],
                                         tpe[:, 128:128 + P_IP])

            # ---- txt k/v projections (5-bank psum, one group per bank) --
            def kv_cloop(kd_, vd_, outp, fill_every):
                for c in range(KC):
                    wk_c = setup_w.tile([128, 1024], f32r, tag="wk1024")
                    n.sync.dma_start(
                        wk_c[:],
                        kd_.rearrange("(co ci) m -> ci co m",
                                      ci=128)[:, c, 0:1024])
                    wv_c = setup_w.tile([128, 1024], f32r, tag="wv1024")
                    n.sync.dma_start(
                        wv_c[:],
                        vd_.rearrange("(co ci) m -> ci co m",
                                      ci=128)[:, c, 0:1024])
                    wt_c = setup_w.tile([128, 512], f32r, tag="wt")
                    n.sync.dma_start(
                        wt_c[:, 0:256],
                        kd_.rearrange("(co ci) m -> ci co m",
                                      ci=128)[:, c, 1024:1280])
                    n.sync.dma_start(
                        wt_c[:, 256:512],
                        vd_.rearrange("(co ci) m -> ci co m",
                                      ci=128)[:, c, 1024:1280])
                    for j in range(2):
                        n.tensor.matmul(outp[:, j, :], encT_or(outp)[:, c, :],
                                        wk_c[:, ts(j, 512)],
                                        start=(c == 0), stop=(c == KC - 1))
                        n.tensor.matmul(outp[:, 3 + j, :], encT_or(outp)[:, c, :],
                                        wv_c[:, ts(j, 512)],
                                        start=(c == 0), stop=(c == KC - 1))
                    n.tensor.matmul(outp[:, 2, :], encT_or(outp)[:, c, :],
                                    wt_c[:],
                                    start=(c == 0), stop=(c == KC - 1))
                    if c % 3 == fill_every:
                        maybe_fill(ident)

            def kslot(p, j):
                return p[:, j, 0:512] if j < 2 else p[:, 2, 0:256]

            def vslot(p, j):
                return p[:, 3 + j, 0:512] if j < 2 else p[:, 2, 256:512]

            def drain_kv(p, prows, ksb_flat, voff):
                for j in range(3):
                    w = min(512, D - j * 512)
                    n.scalar.activation(ksb_flat[:, ds(j * 512, w)],
                                        kslot(p, j), FT.Copy)
                with n.allow_low_precision(reason="v row-sums feed small "
                                           "mean correction"):
                    for j in range(3):
                        h0, nh = j * 8, (4 if j == 2 else 8)
                        vj = vslot(p, j).rearrange("p (h c) -> p h c", c=HD)
                        n.vector.tensor_copy(
                            vcat[voff:voff + prows, ds(h0, nh), 0:HD], vj)
                        n.vector.reduce_sum(
                            vcat[voff:voff + prows, ds(h0, nh),
                                 HD + 1:HD + 2], vj,
                            axis=mybir.AxisListType.X)

            encT_or = lambda p: encT if p.shape[0] == T else ipT
            with tc.tile_pool(name="sps1", bufs=1, space="PSUM") as sps1:
                kvp = sps1.tile([T, 5, 512], f32, tag="kvp")
                kv_cloop(wk_d, wv_d, kvp, 2)
                k_sb = setup.tile([T, KD, 128], f32, tag="ksb")
                drain_kv(kvp, T, k_sb[:].rearrange("p a b -> p (a b)"), 0)
            for dt_ in range(KD):
                tpk = ps_acc.tile([128, 512], f32, tag="acc")
                n.tensor.transpose(tpk[:, 0:T], k_sb[:, dt_, :], ident[:T, :T])
                n.vector.tensor_copy(ktc_sb[:, dt_, 0:T], tpk[:, 0:T])

            # ---- ip k/v (row-major, same 5-slot psum pattern) -------------
            with tc.tile_pool(name="sps2", bufs=1, space="PSUM") as sps2:
                ikvp = sps2.tile([P_IP, 5, 512], f32, tag="ikvp")
                kv_cloop(wkip_d, wvip_d, ikvp, 2)
                kip_sb = setup.tile([P_IP, KD, 128], f32, tag="ksb",
                                    name="kip_sb")
                drain_kv(ikvp, P_IP,
                         kip_sb[:].rearrange("p a b -> p (a b)"), IPOFF)
            for dt_ in range(KD):
                tpk = ps_acc.tile([128, 512], f32, tag="acc")
                n.tensor.transpose(tpk[:, 0:P_IP], kip_sb[:, dt_, :],
                                   ident[:P_IP, :P_IP])
                n.vector.tensor_copy(ktc_sb[:, dt_, IPOFF:CAT],
                                     tpk[:, 0:P_IP])

            # ---- w_out dma + bf16 convert (Pool) --------------------------
            wof = wout_d.rearrange("(ko ki) m -> ki ko m", ki=128)
            for k in range(KD):
                wo_a = setup_w.tile([128, 1024], f32, tag="wk1024")
                n.sync.dma_start(wo_a[:], wof[:, k, 0:1024])
                wo_b = setup_w.tile([128, 256], f32, tag="w256")
                n.sync.dma_start(wo_b[:], wof[:, k, 1024:1280])
                n.gpsimd.tensor_copy(wout_bf[:, k, 0:1024], wo_a[:])
                n.gpsimd.tensor_copy(wout_bf[:, k, 1024:1280], wo_b[:])
                if k % 4 == 3:
                    maybe_fill(ident)
            while fills:
                maybe_fill(ident)

        # ================= main loop =======================================
        ps_sc = ctx.enter_context(tc.tile_pool(name="ps_sc", bufs=2,
                                               space="PSUM"))
        ps_pv = ctx.enter_context(tc.tile_pool(name="ps_pv", bufs=3,
                                               space="PSUM"))

        def emit_scores(b, chunks):
            """20 per-head score tiles + exp (one matmul group per psum bank
            -- the NEFF runtime rejects two sequential groups sharing a
            bank); pop a side-work item every other head."""
            qT = qT_tiles.pop(b)
            pT = []
            for h in range(H):
                p, half = h // 2, h % 2
                sc = ps_sc.tile([CAT, SB], f32, tag="sc")
                n.tensor.matmul(sc[:], ktc_sb[ds(64 * half, 64), p, :],
                                qT[ds(64 * half, 64), p, :],
                                start=True, stop=True)
                pt = lpp.tile([CAT, SB], bf16, tag="pT", name=f"pT{b}_{h}")
                n.scalar.activation(pt[:], sc[:], FT.Exp, scale=SCALE)
                pT.append(pt)
                if h % 2 == 0 and chunks:
                    chunks.pop(0)()
            return pT

        BANKS = [list(range(6 * g, min(H, 6 * g + 6))) for g in range(4)]

        def emit_pv_qp(b, pT2, work):
            """16 PV psum tiles (si-major) with normalize; interleave `work`
            (qp closures + leftover out-chunks). Returns (stats_closure, lat)
            -- the ACT squares + DVE stats chain is deferred so it queues on
            ACT *behind the next block's exps* instead of ahead of them."""
            lat = lp1.tile([128, 2, D], bf16, tag="lat", name=f"lat{b}")
            ipo = lp1.tile([128, 2, D], bf16, tag="ipo", name=f"ipo{b}")
            msum_l = lps.tile([128, 2, H], f32, tag="msl")
            msum_i = lps.tile([128, 2, H], f32, tag="msi")
            recip_l = lps.tile([128, 2, H], f32, tag="rcl")
            recip_i = lps.tile([128, 2, H], f32, tag="rci")
            st = lps.tile([128, 2, 16], f32, tag="st", name=f"st{b}")

            for si in range(2):
                for g, bank in enumerate(BANKS):
                    nb = len(bank)
                    for br in range(2):  # 0 = txt, 1 = ip
                        pv = ps_pv.tile([128, 512], f32, tag="pv")
                        r0 = 0 if br == 0 else IPOFF
                        r1 = T if br == 0 else CAT
                        for bi, h in enumerate(bank):
                            n.tensor.matmul(
                                pv[:, ds(bi * VW, VW)],
                                pT2[h][r0:r1, ts(si, 128)],
                                vcat[r0:r1, h, :], start=True, stop=True,
                                tile_position=(r0, 0))
                        pv3 = pv[:, :nb * VW].rearrange("p (h c) -> p h c",
                                                        c=VW)
                        recip = recip_l if br == 0 else recip_i
                        msum = msum_l if br == 0 else msum_i
                        dest = lat if br == 0 else ipo
                        h0 = bank[0]
                        n.vector.reciprocal(recip[:, si, ds(h0, nb)],
                                            pv3[:, :, HD])
                        n.vector.tensor_mul(msum[:, si, ds(h0, nb)],
                                            pv3[:, :, HD + 1],
                                            recip[:, si, ds(h0, nb)])
                        with n.allow_low_precision(reason="attn out bf16"):
                            n.vector.tensor_tensor(
                                dest[:, si, ds(h0 * HD, nb * HD)].rearrange(
                                    "p (h c) -> p h c", c=HD),
                                pv3[:, :, 0:HD],
                                recip[:, si, ds(h0, nb), None].to_broadcast(
                                    [128, nb, HD]),
                                op=ALU.mult)
                        if work:
                            work.pop(0)()
            while work:
                work.pop(0)()

            def stats():
                emit_stats(b, lat, ipo, msum_l, msum_i, st,
                           pool_combine=(b + FILLD >= NBLK))
                return lat

            return stats, lat

        def emit_stats(b, lat, ipo, msum_l, msum_i, st, pool_combine=False):
            """norm_ipa stats + hs_sum for block b (deferred one phase)."""
            scr = lscr.tile([128, D], bf16, tag="scr", name=f"scr{b}")
            for si in range(2):
                n.vector.reduce_sum(st[:, si, 0:1], msum_l[:, si, :],
                                    axis=mybir.AxisListType.X)
                n.vector.reduce_sum(st[:, si, 1:2], msum_i[:, si, :],
                                    axis=mybir.AxisListType.X)
                with n.allow_low_precision(reason="scratch for accum"):
                    n.scalar.activation(scr[:], lat[:, si, :], FT.Square,
                                        accum_out=st[:, si, 2:3])
                    n.scalar.activation(scr[:], ipo[:, si, :], FT.Square,
                                        accum_out=st[:, si, 3:4])
            n.vector.tensor_scalar_mul(st[:, :, 4:6], st[:, :, 0:2], 1.0 / D)
            n.vector.tensor_mul(st[:, :, 6:8], st[:, :, 4:6], st[:, :, 4:6])
            for si in range(2):
                n.vector.tensor_scalar(out=st[:, si, 8:9], in0=st[:, si, 2:3],
                                       scalar1=1.0 / D, scalar2=st[:, si, 6:7],
                                       op0=ALU.mult, op1=ALU.subtract)
                n.vector.tensor_scalar(out=st[:, si, 9:10], in0=st[:, si, 3:4],
                                       scalar1=1.0 / D, scalar2=st[:, si, 7:8],
                                       op0=ALU.mult, op1=ALU.subtract)
            # std = var * rsqrt(var): fast-inverse-sqrt init + 3 Newton iters
            i32 = mybir.dt.int32
            vv = st[:, :, 8:10]
            yy = st[:, :, 10:12]
            t0 = st[:, :, 12:14]
            n.vector.tensor_scalar(out=yy.bitcast(i32), in0=vv.bitcast(i32),
                                   scalar1=1, scalar2=None,
                                   op0=ALU.logical_shift_right)
            n.vector.tensor_scalar(out=yy.bitcast(i32), in0=yy.bitcast(i32),
                                   scalar1=-1, scalar2=0x5f3759df,
                                   op0=ALU.mult, op1=ALU.add)
            for _ in range(3):
                n.vector.tensor_mul(t0[:], yy[:], yy[:])
                n.vector.tensor_mul(t0[:], t0[:], vv[:])
                n.vector.tensor_scalar(out=t0[:], in0=t0[:], scalar1=-0.5,
                                       scalar2=1.5, op0=ALU.mult, op1=ALU.add)
                n.vector.tensor_mul(yy[:], yy[:], t0[:])
            n.vector.tensor_mul(yy[:], vv[:], yy[:])  # std = var * rsqrt(var)
            for si in range(2):
                n.vector.tensor_scalar_add(st[:, si, 12:13], st[:, si, 11:12],
                                           EPS)
                n.vector.reciprocal(st[:, si, 13:14], st[:, si, 12:13])
                n.vector.tensor_mul(st[:, si, 14:15], st[:, si, 10:11],
                                    st[:, si, 13:14])
                # gneg = alpha*mean_ip - mean_lat
                n.vector.scalar_tensor_tensor(
                    out=st[:, si, 15:16], in0=st[:, si, 5:6],
                    scalar=st[:, si, 14:15], in1=st[:, si, 4:5],
                    op0=ALU.mult, op1=ALU.subtract)
                # hs_sum = lat + alpha*ip - gneg   (bf16). Late blocks run
                # it on Pool (DVE is their bottleneck; Pool is idle).
                with n.allow_low_precision(reason="hs_sum bf16"):
                    if pool_combine:
                        n.gpsimd.tensor_scalar_mul(ipo[:, si, :],
                                                   ipo[:, si, :],
                                                   st[:, si, 14:15])
                        n.gpsimd.tensor_add(lat[:, si, :], lat[:, si, :],
                                            ipo[:, si, :])
                        n.gpsimd.tensor_scalar_sub(lat[:, si, :],
                                                   lat[:, si, :],
                                                   st[:, si, 15:16])
                    else:
                        n.vector.scalar_tensor_tensor(
                            out=lat[:, si, :], in0=ipo[:, si, :],
                            scalar=st[:, si, 14:15], in1=lat[:, si, :],
                            op0=ALU.mult, op1=ALU.add)
                        n.vector.tensor_scalar_sub(lat[:, si, :],
                                                   lat[:, si, :],
                                                   st[:, si, 15:16])

        def emit_tr2(b, lat):
            """hs_sum(b) -> hsT2 via DMA xbar transpose (bf16)."""
            hsT2 = lph2.tile([128, KD, SB], bf16, tag="hsT2", name=f"hsT2{b}")
            for si in range(2):
                n.sync.dma_start(hsT2[:, :, ds(si * 128, 128)],
                                 lat[:, si, :], transpose=True)
            return hsT2

        def make_out_chunks(b, hsT2, act_drain=False):
            """12 half-chunk closures for block b's out-projection.
            (si, j, half): 5 accumulating matmuls each; second half drains
            psum + adds bias into ost, the j==2 half DMAs the si row out.
            act_drain: drain via ACT copy + Pool bias-add instead of DVE
            (for blocks whose iteration has no q-proj work: DVE is the
            bottleneck there, ACT/Pool are idle)."""
            s0 = b * SB
            state = {}

            def mk(si, j, half):
                def go():
                    w = min(512, D - j * 512)
                    if half == 0:
                        state[(si, j)] = ps_acc.tile([128, 512], f32,
                                                     tag="acc",
                                                     name=f"op{b}_{si}_{j}")
                    op = state[(si, j)]
                    for k in range(half * 5, half * 5 + 5):
                        n.tensor.matmul(op[:, :w], hsT2[:, k, ts(si, 128)],
                                        wout_bf[:, k, ds(j * 512, w)],
                                        start=(k == 0), stop=(k == KD - 1))
                    if half == 1:
                        if j == 0:
                            state[si] = lpo.tile([128, D], f32, tag="ost",
                                                 name=f"ost{b}_{si}")
                        ost = state[si]
                        if act_drain:
                            n.scalar.activation(ost[:, ds(j * 512, w)],
                                                op[:, :w], FT.Copy)
                            n.gpsimd.tensor_add(ost[:, ds(j * 512, w)],
                                                ost[:, ds(j * 512, w)],
                                                bias_full[:, ds(j * 512, w)])
                        else:
                            n.vector.tensor_tensor(
                                ost[:, ds(j * 512, w)], op[:, :w],
                                bias_full[:, ds(j * 512, w)], op=ALU.add)
                        if j == 2:
                            n.sync.dma_start(
                                out_d[ds(s0 + si * 128, 128), :], ost[:])
                return go

            return [mk(si, j, half)
                    for si in range(2) for j in range(3) for half in range(2)]

        for _b in range(FILLD, FILLD + 2):
            emit_load(_b)
        emit_fill_start(FILLD)  # setup's PE-variant covered 0..FILLD-1
        chunks = []          # pending out-proj half-chunks (block i-2's)
        lat_prev = None      # hs_sum of block i-1, not yet transposed
        stats_prev = None    # deferred stats closure of block i-1
        qp_pool = []         # qp groups; from iter 8 on, take 6/iter so the
                             # otherwise-starved iters 12-14 keep PE side-work
                             # (qp(b) still always drains before iter b).
        for i in range(NBLK):
            qp_pool += make_qp_closures(i + FILLD)
            take = len(qp_pool) if i < 8 else min(len(qp_pool),
                                                  8 if i < 13 else 5)
            my_qp = qp_pool[:take]
            del qp_pool[:take]
            # unified PE side-work queue: chunk halves first (psum ring slot
            # reuse: a later closure's alloc waits an earlier one's drain),
            # then qp groups. 4 items pace the score phase, rest the PV phase.
            work = chunks + my_qp
            chunks = []
            sc_work = work[:4]
            del work[:4]
            pT2 = emit_scores(i, sc_work)
            if stats_prev is not None:
                stats_prev()
            work = sc_work + work
            stats_prev, lat = emit_pv_qp(i, pT2, work)
            emit_load(i + FILLD + 2)
            if lat_prev is not None:
                hsT2 = emit_tr2(i - 1, lat_prev[1])
                chunks = make_out_chunks(i - 1, hsT2,
                                         act_drain=(i >= 13))
            lat_prev = (i, lat)
            emit_fill_start(i + FILLD + 1)
        # tail: block NBLK-2 chunks + block NBLK-1 stats/tr2/out
        for c in chunks:
            c()
        stats_prev()
        hsT2 = emit_tr2(NBLK - 1, lat_prev[1])
        for c in make_out_chunks(NBLK - 1, hsT2, act_drain=True):
            c()
    nc.compile()
    return nc


def _get_nc():
    if "nc" not in _CACHE:
        _CACHE["nc"] = _build()
    return _CACHE["nc"]


def kernel(**inputs) -> np.ndarray:
    nc = _get_nc()
    f = lambda x: np.ascontiguousarray(np.asarray(x), dtype=np.float32)
    shared = {k: f(inputs[k]) for k in
              ("w_q", "w_k", "w_v", "w_k_ip", "w_v_ip", "w_out", "b_out")}
    hs = f(inputs["hidden_states"])
    enc = f(inputs["encoder_hidden_states"])
    ipx = f(inputs["ip_hidden_states"])
    in_maps = [
        dict(shared, hidden_states=hs[i], encoder_hidden_states=enc[i],
             ip_hidden_states=ipx[i])
        for i in range(8)
    ]
    res = bass_utils.run_bass_kernel_spmd(nc, in_maps, core_ids=list(range(8)))
    return np.stack([res.results[i]["out"] for i in range(8)], axis=0)


if __name__ == "__main__":
    rng = np.random.default_rng(0)
    ins = {
        "hidden_states": rng.standard_normal((B, S, D), dtype=np.float32),
        "encoder_hidden_states": rng.standard_normal((B, T, C), dtype=np.float32),
        "ip_hidden_states": rng.standard_normal((B, P_IP, C), dtype=np.float32),
        "w_q": (rng.standard_normal((D, D), dtype=np.float32) * 0.02),
        "w_k": (rng.standard_normal((C, D), dtype=np.float32) * 0.02),
        "w_v": (rng.standard_normal((C, D), dtype=np.float32) * 0.02),
        "w_k_ip": (rng.standard_normal((C, D), dtype=np.float32) * 0.02),
        "w_v_ip": (rng.standard_normal((C, D), dtype=np.float32) * 0.02),
        "w_out": (rng.standard_normal((D, D), dtype=np.float32) * 0.02),
        "b_out": np.zeros((D,), dtype=np.float32),
    }
    out = kernel(**ins)
    print("out", out.shape, out.dtype, float(np.abs(out).max()))


# revision 64
# speedup vs baseline: 1.0129x; 1.0129x over previous
"""Bass/Tile kernel for nn_CustomCrossAttnProcessor (8-core data-parallel).

Each NeuronCore processes one batch element (B=8 == n_cores).

v2 redesign (vs baseline):
  - All transposes moved off the PE: hs -> hsT and hs_sum -> hsT2 go through
    DMA xbar transpose (bf16, 16x128 tiles); Pool converts hs fp32->bf16.
  - Scores paired: one [CAT,512] psum tile holds two heads (two single-shot
    matmuls), one exp per pair (halves ACT op count on the exp path).
  - Out-projection: bias matmul removed (DVE adds a pre-broadcast bias tile
    during the psum->sbuf drain); 12 half-chunks interleave into the score
    and PV phases of the *second-next* block (software pipeline deepened so
    the PE never waits on the DVE stats chain).
  - PV tiles and q-projection accumulation groups interleave so DVE
    normalize latency hides under PE work.
"""
import sys

for _p in ("/opt/trn_rl_repo",):
    if _p not in sys.path:
        sys.path.append(_p)

from contextlib import ExitStack

import numpy as np

import concourse.bass as bass  # noqa: F401
import concourse.tile as tile
import concourse.mybir as mybir
from concourse import bass_utils, bacc
from concourse.bass import ts, ds
from concourse.masks import make_identity

B, S, D = 8, 4096, 1280
T, P_IP, C = 77, 16, 2048
H, HD = 20, 64
SB = 256            # tokens per s-block
NBLK = S // SB      # 16
SCALE = HD ** -0.5  # 0.125
EPS = 1e-7
KD = D // 128       # 10
KC = C // 128       # 16
CAT = 112           # rows: txt [0:77], gap [77:96], ip [96:112]
IPOFF = 96
VW = HD + 2         # 66: v cols + ones col (softmax sum) + v-rowsum col
FILLD = 3           # q-proj pipeline depth
ALU = mybir.AluOpType
FT = mybir.ActivationFunctionType

f32 = mybir.dt.float32
f32r = mybir.dt.float32r
bf16 = mybir.dt.bfloat16

_CACHE = {}


def _build():
    nc = bacc.Bacc(
        "TRN2", target_bir_lowering=False, debug=False, enable_asserts=False,
        num_devices=8,
    )
    hs_d = nc.dram_tensor("hidden_states", [S, D], f32, kind="ExternalInput").ap()
    enc_d = nc.dram_tensor("encoder_hidden_states", [T, C], f32,
                           kind="ExternalInput").ap()
    ip_d = nc.dram_tensor("ip_hidden_states", [P_IP, C], f32,
                          kind="ExternalInput").ap()
    wq_d = nc.dram_tensor("w_q", [D, D], f32r, kind="ExternalInput").ap()
    wk_d = nc.dram_tensor("w_k", [C, D], f32r, kind="ExternalInput").ap()
    wv_d = nc.dram_tensor("w_v", [C, D], f32r, kind="ExternalInput").ap()
    wkip_d = nc.dram_tensor("w_k_ip", [C, D], f32r, kind="ExternalInput").ap()
    wvip_d = nc.dram_tensor("w_v_ip", [C, D], f32r, kind="ExternalInput").ap()
    wout_d = nc.dram_tensor("w_out", [D, D], f32, kind="ExternalInput").ap()
    bout_d = nc.dram_tensor("b_out", [D], f32, kind="ExternalInput").ap()
    out_d = nc.dram_tensor("out", [S, D], f32, kind="ExternalOutput").ap()

    with tile.TileContext(nc) as tc, ExitStack() as ctx:
        n = tc.nc
        const = ctx.enter_context(tc.tile_pool(name="const", bufs=1))
        wq_sb = const.tile([128, KD, D], bf16)
        wout_bf = const.tile([128, KD, D], bf16)
        ktc_sb = const.tile([128, KD, CAT], bf16)
        vcat = const.tile([CAT, H, VW], bf16)
        bias_full = const.tile([128, D], f32)

        n.gpsimd.memset(ktc_sb[:, :, T:IPOFF], 0.0)
        n.vector.memset(vcat[0:T, :, HD:HD + 1], 1.0)
        n.vector.memset(vcat[IPOFF:CAT, :, HD:HD + 1], 1.0)

        # ---------------- loop pools -------------------------------------
        lp = ctx.enter_context(tc.tile_pool(name="lp", bufs=3))      # hs f32
        lpb = ctx.enter_context(tc.tile_pool(name="lpb", bufs=2))    # hs bf16
        lph = ctx.enter_context(tc.tile_pool(name="lph", bufs=2))    # hsT
        lpq = ctx.enter_context(tc.tile_pool(name="lpq", bufs=FILLD))  # qT
        lscr = ctx.enter_context(tc.tile_pool(name="lscr", bufs=1))  # sq scratch
        lp1 = ctx.enter_context(tc.tile_pool(name="lp1", bufs=2))    # lat/ipo
        lph2 = ctx.enter_context(tc.tile_pool(name="lph2", bufs=2))  # hsT2
        lps = ctx.enter_context(tc.tile_pool(name="lps", bufs=2))    # stats
        lpo = ctx.enter_context(tc.tile_pool(name="lpo", bufs=2))    # ost
        lpp = ctx.enter_context(tc.tile_pool(name="lpp", bufs=20))   # pT
        # psum: acc ring ([128,512]: q-proj groups, out-proj chunks, setup
        # transposes). scores + pv rings alloc'd after setup (bank budget:
        # setup kvp needs 5 banks alongside acc's 3).
        ps_acc = ctx.enter_context(tc.tile_pool(name="ps_acc", bufs=2,
                                                space="PSUM"))

        hs_tiles = {}

        def emit_load(b):
            if b >= NBLK:
                return
            for si in range(2):
                t_ = lp.tile([128, D], f32, tag="hs", name=f"hs{b}_{si}")
                n.sync.dma_start(t_[:], hs_d[ds(b * SB + si * 128, 128), :])
                hs_tiles[(b, si)] = t_

        qT_tiles = {}
        hsT_tiles = {}

        def emit_fill_start(b):
            """hs fp32 -> bf16 (Pool), then DMA xbar transpose -> hsT."""
            if b >= NBLK:
                return
            hsT = lph.tile([128, KD, SB], bf16, tag="hsT", name=f"hsT{b}")
            for si in range(2):
                hsb = lpb.tile([128, D], bf16, tag="hsb")
                n.gpsimd.tensor_copy(hsb[:], hs_tiles.pop((b, si))[:])
                n.sync.dma_start(hsT[:, :, ds(si * 128, 128)], hsb[:],
                                 transpose=True)
            hsT_tiles[b] = hsT

        def make_qp_closures(b):
            """10 closures: q-projection for block b in (dp, dd) groups of
            10 accumulating matmuls each; qT drain (ACT) after each dd pair."""
            if b >= NBLK:
                return []
            hsT = hsT_tiles.pop(b)
            qT = lpq.tile([128, KD, SB], bf16, tag="qT", name=f"qT{b}")
            qT_tiles[b] = qT
            out = []
            state = {}

            def mk(dp, dd):
                def go():
                    if dd == 0:
                        state["qp"] = ps_acc.tile([128, 512], f32, tag="acc",
                                                  name=f"qp{b}_{dp}")
                    qp = state["qp"]
                    for k in range(KD):
                        n.tensor.matmul(qp[:, ds(dd * SB, SB)],
                                        wq_sb[:, k, ts(dp + dd, 128)],
                                        hsT[:, k, :], start=(k == 0),
                                        stop=(k == KD - 1))
                    if dd == 1:
                        n.scalar.activation(
                            qT[:, dp:dp + 2, :].rearrange("p a b -> p (a b)"),
                            qp[:], FT.Copy)
                return go

            for dp in range(0, KD, 2):
                for dd in range(2):
                    out.append(mk(dp, dd))
            return out

        def emit_fill_start_pe(b, ident):
            """setup-only fill start: PE transposes of fp32 hs (PE is idle
            during the DMA-bound setup; keeps the fill off the serial DMA
            queue, where a not-yet-ready xbar transpose head-of-line blocks
            the weight stream)."""
            hsT = lph.tile([128, KD, SB], bf16, tag="hsT", name=f"hsT{b}")
            for si in range(2):
                hst = hs_tiles.pop((b, si))
                for g0 in range(0, KD, 4):
                    kk = min(4, KD - g0)
                    tp = ps_acc.tile([128, 512], f32, tag="acc",
                                     name=f"tp{b}_{si}_{g0}")
                    for j in range(kk):
                        n.tensor.transpose(tp[:, ds(j * 128, 128)],
                                           hst[:, ts(g0 + j, 128)], ident[:])
                    n.scalar.activation(
                        hsT[:, g0:g0 + kk, ds(si * 128, 128)],
                        tp[:, 0:kk * 128].rearrange("p (a b) -> p a b", b=128),
                        FT.Copy)
            hsT_tiles[b] = hsT

        def emit_fill(b, ident):
            """setup-time fill: PE-transpose start + all qp groups inline."""
            if b >= NBLK:
                return
            emit_fill_start_pe(b, ident)
            for c in make_qp_closures(b):
                c()

        fills = list(range(FILLD))

        def maybe_fill(ident):
            if fills:
                b = fills.pop(0)
                emit_fill(b, ident)
                if b + 2 < FILLD:
                    emit_load(b + 2)  # later loads defer past the weights

        # ================= setup ===========================================
        with tc.tile_pool(name="setup", bufs=1) as setup, \
             tc.tile_pool(name="setup_w", bufs=2) as setup_w:
            ident = setup.tile([128, 128], f32, tag="ident")
            make_identity(n, ident[:])
            ones_bf = setup.tile([1, 128], bf16, tag="ones")
            bias_bf = setup.tile([1, D], bf16, tag="biasbf")
            n.gpsimd.memset(ones_bf[:], 1.0)
            encT = setup.tile([128, KC, T], f32r, tag="encT")
            ipT = setup.tile([128, KC, P_IP], f32r, tag="ipT")
            with tc.tile_pool(name="setup_e", bufs=1) as setup_e:
                enc_sb = setup_e.tile([CAT, C], f32, tag="enc")
                n.sync.dma_start(enc_sb[0:T, :], enc_d)
                n.sync.dma_start(enc_sb[IPOFF:CAT, :], ip_d)
                n.sync.dma_start(bias_full[0:1, :], bout_d[None, :])
                # stage + convert w_q chunks to bf16 (Pool)
                wqf = wq_d.bitcast(f32).rearrange("(ko ki) m -> ki ko m",
                                                  ki=128)
                for k in range(KD):
                    wq_a = setup_w.tile([128, 1024], f32, tag="wk1024")
                    n.sync.dma_start(wq_a[:], wqf[:, k, 0:1024])
                    wq_b = setup_w.tile([128, 256], f32, tag="w256")
                    n.sync.dma_start(wq_b[:], wqf[:, k, 1024:1280])
                    n.gpsimd.tensor_copy(wq_sb[:, k, 0:1024], wq_a[:])
                    n.gpsimd.tensor_copy(wq_sb[:, k, 1024:1280], wq_b[:])
                emit_load(0)
                emit_load(1)
                n.gpsimd.tensor_copy(bias_bf[:], bias_full[0:1, :])
                # bias broadcast to all 128 partitions via PE outer product
                for j in range(3):
                    w = min(512, D - j * 512)
                    pb = ps_acc.tile([128, 512], f32, tag="acc")
                    n.tensor.matmul(pb[:, :w], ones_bf[:],
                                    bias_bf[:, ds(j * 512, w)],
                                    start=True, stop=True)
                    n.vector.tensor_copy(bias_full[:, ds(j * 512, w)],
                                         pb[:, :w])
                for c in range(KC):
                    tpe = ps_acc.tile([128, 512], f32, tag="acc")
                    n.tensor.transpose(tpe[:, 0:T], enc_sb[0:T, ts(c, 128)],
                                       ident[:T, :T])
                    n.tensor.transpose(tpe[:, 128:128 + P_IP],
                                       enc_sb[IPOFF:CAT, ts(c, 128)],
                                       ident[IPOFF:CAT, IPOFF:IPOFF + P_IP],
                                       tile_position=(IPOFF, 0))
                    n.vector.tensor_copy(encT[:, c, :], tpe[:, 0:T])
                    n.vector.tensor_copy(ipT[:, c, :],
                                         tpe[:, 128:128 + P_IP])

            # ---- txt k/v projections (5-bank psum, one group per bank) --
            def kv_cloop(kd_, vd_, outp, fill_every):
                for c in range(KC):
                    wk_c = setup_w.tile([128, 1024], f32r, tag="wk1024")
                    n.sync.dma_start(
                        wk_c[:],
                        kd_.rearrange("(co ci) m -> ci co m",
                                      ci=128)[:, c, 0:1024])
                    wv_c = setup_w.tile([128, 1024], f32r, tag="wv1024")
                    n.sync.dma_start(
                        wv_c[:],
                        vd_.rearrange("(co ci) m -> ci co m",
                                      ci=128)[:, c, 0:1024])
                    wt_c = setup_w.tile([128, 512], f32r, tag="wt")
                    n.sync.dma_start(
                        wt_c[:, 0:256],
                        kd_.rearrange("(co ci) m -> ci co m",
                                      ci=128)[:, c, 1024:1280])
                    n.sync.dma_start(
                        wt_c[:, 256:512],
                        vd_.rearrange("(co ci) m -> ci co m",
                                      ci=128)[:, c, 1024:1280])
                    for j in range(2):
                        n.tensor.matmul(outp[:, j, :], encT_or(outp)[:, c, :],
                                        wk_c[:, ts(j, 512)],
                                        start=(c == 0), stop=(c == KC - 1))
                        n.tensor.matmul(outp[:, 3 + j, :], encT_or(outp)[:, c, :],
                                        wv_c[:, ts(j, 512)],
                                        start=(c == 0), stop=(c == KC - 1))
                    n.tensor.matmul(outp[:, 2, :], encT_or(outp)[:, c, :],
                                    wt_c[:],
                                    start=(c == 0), stop=(c == KC - 1))
                    if c % 3 == fill_every:
                        maybe_fill(ident)

            def kslot(p, j):
                return p[:, j, 0:512] if j < 2 else p[:, 2, 0:256]

            def vslot(p, j):
                return p[:, 3 + j, 0:512] if j < 2 else p[:, 2, 256:512]

            def drain_kv(p, prows, ksb_flat, voff):
                for j in range(3):
                    w = min(512, D - j * 512)
                    n.scalar.activation(ksb_flat[:, ds(j * 512, w)],
                                        kslot(p, j), FT.Copy)
                with n.allow_low_precision(reason="v row-sums feed small "
                                           "mean correction"):
                    for j in range(3):
                        h0, nh = j * 8, (4 if j == 2 else 8)
                        vj = vslot(p, j).rearrange("p (h c) -> p h c", c=HD)
                        n.vector.tensor_copy(
                            vcat[voff:voff + prows, ds(h0, nh), 0:HD], vj)
                        n.vector.reduce_sum(
                            vcat[voff:voff + prows, ds(h0, nh),
                                 HD + 1:HD + 2], vj,
                            axis=mybir.AxisListType.X)

            encT_or = lambda p: encT if p.shape[0] == T else ipT
            with tc.tile_pool(name="sps1", bufs=1, space="PSUM") as sps1:
                kvp = sps1.tile([T, 5, 512], f32, tag="kvp")
                kv_cloop(wk_d, wv_d, kvp, 2)
                k_sb = setup.tile([T, KD, 128], f32, tag="ksb")
                drain_kv(kvp, T, k_sb[:].rearrange("p a b -> p (a b)"), 0)
            for dt_ in range(KD):
                tpk = ps_acc.tile([128, 512], f32, tag="acc")
                n.tensor.transpose(tpk[:, 0:T], k_sb[:, dt_, :], ident[:T, :T])
                n.vector.tensor_copy(ktc_sb[:, dt_, 0:T], tpk[:, 0:T])

            # ---- ip k/v (row-major, same 5-slot psum pattern) -------------
            with tc.tile_pool(name="sps2", bufs=1, space="PSUM") as sps2:
                ikvp = sps2.tile([P_IP, 5, 512], f32, tag="ikvp")
                kv_cloop(wkip_d, wvip_d, ikvp, 2)
                kip_sb = setup.tile([P_IP, KD, 128], f32, tag="ksb",
                                    name="kip_sb")
                drain_kv(ikvp, P_IP,
                         kip_sb[:].rearrange("p a b -> p (a b)"), IPOFF)
            for dt_ in range(KD):
                tpk = ps_acc.tile([128, 512], f32, tag="acc")
                n.tensor.transpose(tpk[:, 0:P_IP], kip_sb[:, dt_, :],
                                   ident[:P_IP, :P_IP])
                n.vector.tensor_copy(ktc_sb[:, dt_, IPOFF:CAT],
                                     tpk[:, 0:P_IP])

            # ---- w_out dma + bf16 convert (Pool) --------------------------
            wof = wout_d.rearrange("(ko ki) m -> ki ko m", ki=128)
            for k in range(KD):
                wo_a = setup_w.tile([128, 1024], f32, tag="wk1024")
                n.sync.dma_start(wo_a[:], wof[:, k, 0:1024])
                wo_b = setup_w.tile([128, 256], f32, tag="w256")
                n.sync.dma_start(wo_b[:], wof[:, k, 1024:1280])
                n.gpsimd.tensor_copy(wout_bf[:, k, 0:1024], wo_a[:])
                n.gpsimd.tensor_copy(wout_bf[:, k, 1024:1280], wo_b[:])
                if k % 4 == 3:
                    maybe_fill(ident)
            while fills:
                maybe_fill(ident)

        # ================= main loop =======================================
        ps_sc = ctx.enter_context(tc.tile_pool(name="ps_sc", bufs=3,
                                               space="PSUM"))
        ps_pv = ctx.enter_context(tc.tile_pool(name="ps_pv", bufs=3,
                                               space="PSUM"))

        def emit_scores(b, chunks):
            """20 per-head score tiles + exp (one matmul group per psum bank
            -- the NEFF runtime rejects two sequential groups sharing a
            bank); pop a side-work item every other head."""
            qT = qT_tiles.pop(b)
            pT = []
            for h in range(H):
                p, half = h // 2, h % 2
                sc = ps_sc.tile([CAT, SB], f32, tag="sc")
                n.tensor.matmul(sc[:], ktc_sb[ds(64 * half, 64), p, :],
                                qT[ds(64 * half, 64), p, :],
                                start=True, stop=True)
                pt = lpp.tile([CAT, SB], bf16, tag="pT", name=f"pT{b}_{h}")
                n.scalar.activation(pt[:], sc[:], FT.Exp, scale=SCALE)
                pT.append(pt)
                if h % 2 == 0 and chunks:
                    chunks.pop(0)()
            return pT

        BANKS = [list(range(6 * g, min(H, 6 * g + 6))) for g in range(4)]

        def emit_pv_qp(b, pT2, work):
            """16 PV psum tiles (si-major) with normalize; interleave `work`
            (qp closures + leftover out-chunks). Returns (stats_closure, lat)
            -- the ACT squares + DVE stats chain is deferred so it queues on
            ACT *behind the next block's exps* instead of ahead of them."""
            lat = lp1.tile([128, 2, D], bf16, tag="lat", name=f"lat{b}")
            ipo = lp1.tile([128, 2, D], bf16, tag="ipo", name=f"ipo{b}")
            msum_l = lps.tile([128, 2, H], f32, tag="msl")
            msum_i = lps.tile([128, 2, H], f32, tag="msi")
            recip_l = lps.tile([128, 2, H], f32, tag="rcl")
            recip_i = lps.tile([128, 2, H], f32, tag="rci")
            st = lps.tile([128, 2, 16], f32, tag="st", name=f"st{b}")

            for si in range(2):
                for g, bank in enumerate(BANKS):
                    nb = len(bank)
                    for br in range(2):  # 0 = txt, 1 = ip
                        pv = ps_pv.tile([128, 512], f32, tag="pv")
                        r0 = 0 if br == 0 else IPOFF
                        r1 = T if br == 0 else CAT
                        for bi, h in enumerate(bank):
                            n.tensor.matmul(
                                pv[:, ds(bi * VW, VW)],
                                pT2[h][r0:r1, ts(si, 128)],
                                vcat[r0:r1, h, :], start=True, stop=True,
                                tile_position=(r0, 0))
                        pv3 = pv[:, :nb * VW].rearrange("p (h c) -> p h c",
                                                        c=VW)
                        recip = recip_l if br == 0 else recip_i
                        msum = msum_l if br == 0 else msum_i
                        dest = lat if br == 0 else ipo
                        h0 = bank[0]
                        n.vector.reciprocal(recip[:, si, ds(h0, nb)],
                                            pv3[:, :, HD])
                        n.vector.tensor_mul(msum[:, si, ds(h0, nb)],
                                            pv3[:, :, HD + 1],
                                            recip[:, si, ds(h0, nb)])
                        with n.allow_low_precision(reason="attn out bf16"):
                            n.vector.tensor_tensor(
                                dest[:, si, ds(h0 * HD, nb * HD)].rearrange(
                                    "p (h c) -> p h c", c=HD),
                                pv3[:, :, 0:HD],
                                recip[:, si, ds(h0, nb), None].to_broadcast(
                                    [128, nb, HD]),
                                op=ALU.mult)
                        if work:
                            work.pop(0)()
            while work:
                work.pop(0)()

            def stats():
                emit_stats(b, lat, ipo, msum_l, msum_i, st,
                           pool_combine=(b + FILLD >= NBLK))
                return lat

            return stats, lat

        def emit_stats(b, lat, ipo, msum_l, msum_i, st, pool_combine=False):
            """norm_ipa stats + hs_sum for block b (deferred one phase)."""
            scr = lscr.tile([128, D], bf16, tag="scr", name=f"scr{b}")
            for si in range(2):
                n.vector.reduce_sum(st[:, si, 0:1], msum_l[:, si, :],
                                    axis=mybir.AxisListType.X)
                n.vector.reduce_sum(st[:, si, 1:2], msum_i[:, si, :],
                                    axis=mybir.AxisListType.X)
                with n.allow_low_precision(reason="scratch for accum"):
                    n.scalar.activation(scr[:], lat[:, si, :], FT.Square,
                                        accum_out=st[:, si, 2:3])
                    n.scalar.activation(scr[:], ipo[:, si, :], FT.Square,
                                        accum_out=st[:, si, 3:4])
            n.vector.tensor_scalar_mul(st[:, :, 4:6], st[:, :, 0:2], 1.0 / D)
            n.vector.tensor_mul(st[:, :, 6:8], st[:, :, 4:6], st[:, :, 4:6])
            for si in range(2):
                n.vector.tensor_scalar(out=st[:, si, 8:9], in0=st[:, si, 2:3],
                                       scalar1=1.0 / D, scalar2=st[:, si, 6:7],
                                       op0=ALU.mult, op1=ALU.subtract)
                n.vector.tensor_scalar(out=st[:, si, 9:10], in0=st[:, si, 3:4],
                                       scalar1=1.0 / D, scalar2=st[:, si, 7:8],
                                       op0=ALU.mult, op1=ALU.subtract)
            # std = var * rsqrt(var): fast-inverse-sqrt init + 3 Newton iters
            i32 = mybir.dt.int32
            vv = st[:, :, 8:10]
            yy = st[:, :, 10:12]
            t0 = st[:, :, 12:14]
            n.vector.tensor_scalar(out=yy.bitcast(i32), in0=vv.bitcast(i32),
                                   scalar1=1, scalar2=None,
                                   op0=ALU.logical_shift_right)
            n.vector.tensor_scalar(out=yy.bitcast(i32), in0=yy.bitcast(i32),
                                   scalar1=-1, scalar2=0x5f3759df,
                                   op0=ALU.mult, op1=ALU.add)
            for _ in range(3):
                n.vector.tensor_mul(t0[:], yy[:], yy[:])
                n.vector.tensor_mul(t0[:], t0[:], vv[:])
                n.vector.tensor_scalar(out=t0[:], in0=t0[:], scalar1=-0.5,
                                       scalar2=1.5, op0=ALU.mult, op1=ALU.add)
                n.vector.tensor_mul(yy[:], yy[:], t0[:])
            n.vector.tensor_mul(yy[:], vv[:], yy[:])  # std = var * rsqrt(var)
            for si in range(2):
                n.vector.tensor_scalar_add(st[:, si, 12:13], st[:, si, 11:12],
                                           EPS)
                n.vector.reciprocal(st[:, si, 13:14], st[:, si, 12:13])
                n.vector.tensor_mul(st[:, si, 14:15], st[:, si, 10:11],
                                    st[:, si, 13:14])
                # gneg = alpha*mean_ip - mean_lat
                n.vector.scalar_tensor_tensor(
                    out=st[:, si, 15:16], in0=st[:, si, 5:6],
                    scalar=st[:, si, 14:15], in1=st[:, si, 4:5],
                    op0=ALU.mult, op1=ALU.subtract)
                # hs_sum = lat + alpha*ip - gneg   (bf16). Late blocks run
                # it on Pool (DVE is their bottleneck; Pool is idle).
                with n.allow_low_precision(reason="hs_sum bf16"):
                    if pool_combine:
                        n.gpsimd.tensor_scalar_mul(ipo[:, si, :],
                                                   ipo[:, si, :],
                                                   st[:, si, 14:15])
                        n.gpsimd.tensor_add(lat[:, si, :], lat[:, si, :],
                                            ipo[:, si, :])
                        n.gpsimd.tensor_scalar_sub(lat[:, si, :],
                                                   lat[:, si, :],
                                                   st[:, si, 15:16])
                    else:
                        n.vector.scalar_tensor_tensor(
                            out=lat[:, si, :], in0=ipo[:, si, :],
                            scalar=st[:, si, 14:15], in1=lat[:, si, :],
                            op0=ALU.mult, op1=ALU.add)
                        n.vector.tensor_scalar_sub(lat[:, si, :],
                                                   lat[:, si, :],
                                                   st[:, si, 15:16])

        def emit_tr2(b, lat):
            """hs_sum(b) -> hsT2 via DMA xbar transpose (bf16)."""
            hsT2 = lph2.tile([128, KD, SB], bf16, tag="hsT2", name=f"hsT2{b}")
            for si in range(2):
                n.sync.dma_start(hsT2[:, :, ds(si * 128, 128)],
                                 lat[:, si, :], transpose=True)
            return hsT2

        def make_out_chunks(b, hsT2, act_drain=False):
            """12 half-chunk closures for block b's out-projection.
            (si, j, half): 5 accumulating matmuls each; second half drains
            psum + adds bias into ost, the j==2 half DMAs the si row out.
            act_drain: drain via ACT copy + Pool bias-add instead of DVE
            (for blocks whose iteration has no q-proj work: DVE is the
            bottleneck there, ACT/Pool are idle)."""
            s0 = b * SB
            state = {}

            def mk(si, j, half):
                def go():
                    w = min(512, D - j * 512)
                    if half == 0:
                        state[(si, j)] = ps_acc.tile([128, 512], f32,
                                                     tag="acc",
                                                     name=f"op{b}_{si}_{j}")
                    op = state[(si, j)]
                    for k in range(half * 5, half * 5 + 5):
                        n.tensor.matmul(op[:, :w], hsT2[:, k, ts(si, 128)],
                                        wout_bf[:, k, ds(j * 512, w)],
                                        start=(k == 0), stop=(k == KD - 1))
                    if half == 1:
                        if j == 0:
                            state[si] = lpo.tile([128, D], f32, tag="ost",
                                                 name=f"ost{b}_{si}")
                        ost = state[si]
                        if act_drain:
                            n.scalar.activation(ost[:, ds(j * 512, w)],
                                                op[:, :w], FT.Copy)
                            n.gpsimd.tensor_add(ost[:, ds(j * 512, w)],
                                                ost[:, ds(j * 512, w)],
                                                bias_full[:, ds(j * 512, w)])
                        else:
                            n.vector.tensor_tensor(
                                ost[:, ds(j * 512, w)], op[:, :w],
                                bias_full[:, ds(j * 512, w)], op=ALU.add)
                        if j == 2:
                            n.sync.dma_start(
                                out_d[ds(s0 + si * 128, 128), :], ost[:])
                return go

            return [mk(si, j, half)
                    for si in range(2) for j in range(3) for half in range(2)]

        for _b in range(FILLD, FILLD + 2):
            emit_load(_b)
        emit_fill_start(FILLD)  # setup's PE-variant covered 0..FILLD-1
        chunks = []          # pending out-proj half-chunks (block i-2's)
        lat_prev = None      # hs_sum of block i-1, not yet transposed
        stats_prev = None    # deferred stats closure of block i-1
        qp_pool = []         # qp groups; from iter 8 on, take 6/iter so the
                             # otherwise-starved iters 12-14 keep PE side-work
                             # (qp(b) still always drains before iter b).
        for i in range(NBLK):
            qp_pool += make_qp_closures(i + FILLD)
            take = len(qp_pool) if i < 8 else min(len(qp_pool),
                                                  8 if i < 13 else 5)
            my_qp = qp_pool[:take]
            del qp_pool[:take]
            # unified PE side-work queue: chunk halves first (psum ring slot
            # reuse: a later closure's alloc waits an earlier one's drain),
            # then qp groups. 4 items pace the score phase, rest the PV phase.
            work = chunks + my_qp
            chunks = []
            sc_work = work[:4]
            del work[:4]
            pT2 = emit_scores(i, sc_work)
            if stats_prev is not None:
                stats_prev()
            work = sc_work + work
            stats_prev, lat = emit_pv_qp(i, pT2, work)
            emit_load(i + FILLD + 2)
            if lat_prev is not None:
                hsT2 = emit_tr2(i - 1, lat_prev[1])
                chunks = make_out_chunks(i - 1, hsT2,
                                         act_drain=(i >= 13))
            lat_prev = (i, lat)
            emit_fill_start(i + FILLD + 1)
        # tail: block NBLK-2 chunks + block NBLK-1 stats/tr2/out
        for c in chunks:
            c()
        stats_prev()
        hsT2 = emit_tr2(NBLK - 1, lat_prev[1])
        for c in make_out_chunks(NBLK - 1, hsT2, act_drain=True):
            c()
    nc.compile()
    return nc


def _get_nc():
    if "nc" not in _CACHE:
        _CACHE["nc"] = _build()
    return _CACHE["nc"]


def kernel(**inputs) -> np.ndarray:
    nc = _get_nc()
    f = lambda x: np.ascontiguousarray(np.asarray(x), dtype=np.float32)
    shared = {k: f(inputs[k]) for k in
              ("w_q", "w_k", "w_v", "w_k_ip", "w_v_ip", "w_out", "b_out")}
    hs = f(inputs["hidden_states"])
    enc = f(inputs["encoder_hidden_states"])
    ipx = f(inputs["ip_hidden_states"])
    in_maps = [
        dict(shared, hidden_states=hs[i], encoder_hidden_states=enc[i],
             ip_hidden_states=ipx[i])
        for i in range(8)
    ]
    res = bass_utils.run_bass_kernel_spmd(nc, in_maps, core_ids=list(range(8)))
    return np.stack([res.results[i]["out"] for i in range(8)], axis=0)


if __name__ == "__main__":
    rng = np.random.default_rng(0)
    ins = {
        "hidden_states": rng.standard_normal((B, S, D), dtype=np.float32),
        "encoder_hidden_states": rng.standard_normal((B, T, C), dtype=np.float32),
        "ip_hidden_states": rng.standard_normal((B, P_IP, C), dtype=np.float32),
        "w_q": (rng.standard_normal((D, D), dtype=np.float32) * 0.02),
        "w_k": (rng.standard_normal((C, D), dtype=np.float32) * 0.02),
        "w_v": (rng.standard_normal((C, D), dtype=np.float32) * 0.02),
        "w_k_ip": (rng.standard_normal((C, D), dtype=np.float32) * 0.02),
        "w_v_ip": (rng.standard_normal((C, D), dtype=np.float32) * 0.02),
        "w_out": (rng.standard_normal((D, D), dtype=np.float32) * 0.02),
        "b_out": np.zeros((D,), dtype=np.float32),
    }
    out = kernel(**ins)
    print("out", out.shape, out.dtype, float(np.abs(out).max()))
